# revision 5
# baseline (speedup 1.0000x reference)
"""DepthCueExtractor TRN2 kernel (bf16 I/O).

out[b,u,y,x,n] = mean_v(lfi[b,u,y,x,v]) * s_mask[b,n] * h_mask[b,n,y]
  s_mask[b,n]   = sum_{h,w} f_maps[b,h,w,n]
  h_mask[b,n,y] = colsum[b,y,n] / max_w colsum[b,w,n]
  colsum[b,w,n] = sum_h f_maps[b,h,w,n]

Sharding: 8 cores = (batch b in 0..3) x (H-half in 0..1), data-parallel on the
output. Each core reads its lfi slice plus only its 128-column W-half of
f_maps[b]; the pair (2b, 2b+1) exchanges 512B of partial colsum stats
(sum/max over its half) via an in-pair AllGather, so no f_maps bytes are read
twice.

All HBM traffic is bf16 (host down/up-casts around the device call): the
rel-err tolerance of the problem (2e-2) is ~25x above the ~8e-3 worst-case
quantization error, and it halves the DMA-bound runtime vs fp32. colsum is
reduced entirely on the PE (pairs of accumulating ones-matmuls, no DVE
pre-add). The output phase multiplies mlf[y,x] (broadcast over the OUTER n
dim, innermost x contiguous) against a materialized n-major expansion
wfx[y,n,x] of the per-(y,n) weight, which keeps every operand 2-byte +
innermost-contiguous so the DVE runs in its 2x mode; the expansion itself is
built by log2(W) doubling copies that run in the DVE 4x copy mode. Stores
iterate (x,n) on the SBUF side so the HBM side stays 8KB-contiguous per
partition. ~47.25MB of HBM traffic per core at 360GB/s aggregate."""

import numpy as np
from ml_dtypes import bfloat16

import concourse.bass as bass
import concourse.bacc as bacc
import concourse.bass_isa as bass_isa
import concourse.mybir as mybir
import concourse.tile as tile
from concourse.bass_utils import run_bass_kernel_spmd

F32 = mybir.dt.float32
BF16 = mybir.dt.bfloat16

B, U, H, W, V, N = 4, 9, 256, 256, 9, 64
HY = H // 2

REPLICA_GROUPS = [[0, 1], [2, 3], [4, 5], [6, 7]]


def build_kernel_body(nc, tc, lfi_s, fm, out_s, cc_in, cc_out):
    with (
        tc.tile_pool(name="const", bufs=1) as const_pool,
        tc.tile_pool(name="fmp", bufs=2) as fm_pool,
        tc.tile_pool(name="psum", bufs=1, space="PSUM") as psum_pool,
        tc.tile_pool(name="stats", bufs=1) as stats_pool,
        tc.tile_pool(name="lfip", bufs=1) as lfi_pool,
        tc.tile_pool(name="mlfp", bufs=1) as mlf_pool,
        tc.tile_pool(name="wfxp", bufs=1) as wfx_pool,
        tc.tile_pool(name="outp", bufs=2) as out_pool,
    ):
        ones = const_pool.tile([128, 1], BF16)
        nc.vector.memset(ones[:], 1.0)

        # ---- Phase A: colsum[w, n] = sum_h fm[h, w, n] for my 128 w's.
        # Reduced on the PE alone: per (w-quarter, n) two accumulating
        # matmuls contract the two 128-row h-halves.
        WQ = 64  # w-chunk width (PE out base partition must be 0/32/64)
        cs_psum = psum_pool.tile([128, N], F32)
        for wq in range(128 // WQ):
            sl = slice(wq * WQ, (wq + 1) * WQ)
            f0 = fm_pool.tile([128, WQ, N], BF16, name=f"f0_{wq}", tag="f0", bufs=2)
            f1 = fm_pool.tile([128, WQ, N], BF16, name=f"f1_{wq}", tag="f1", bufs=2)
            nc.sync.dma_start(out=f0[:], in_=fm[0:128, sl, :])
            nc.sync.dma_start(out=f1[:], in_=fm[128:256, sl, :])
            for n in range(N):
                nc.tensor.matmul(
                    out=cs_psum[sl, n : n + 1],
                    lhsT=f0[:, :, n],
                    rhs=ones[:, 0:1],
                    start=True,
                    stop=False,
                )
                nc.tensor.matmul(
                    out=cs_psum[sl, n : n + 1],
                    lhsT=f1[:, :, n],
                    rhs=ones[:, 0:1],
                    start=False,
                    stop=True,
                )

        hp = tc.high_priority
        with hp():
            cs_sb = stats_pool.tile([128, N], F32)
            nc.vector.tensor_copy(out=cs_sb[:], in_=cs_psum[:])

        # ---- Phase A2: partial stats over my half, exchange via AllGather.
        with hp():
            red_s = stats_pool.tile([128, N], F32)
            nc.gpsimd.partition_all_reduce(
                red_s[:], cs_sb[:], 128, bass_isa.ReduceOp.add
            )
            red_m = stats_pool.tile([128, N], F32)
            nc.gpsimd.partition_all_reduce(
                red_m[:], cs_sb[:], 128, bass_isa.ReduceOp.max
            )

            pack = stats_pool.tile([1, 2 * N], F32)
            nc.vector.tensor_copy(out=pack[0:1, 0:N], in_=red_s[0:1, :])
            nc.vector.tensor_copy(out=pack[0:1, N : 2 * N], in_=red_m[0:1, :])
            nc.sync.dma_start(out=cc_in[:], in_=pack[0:1, :])

            nc.gpsimd.collective_compute(
                "AllGather",
                mybir.AluOpType.bypass,
                replica_groups=REPLICA_GROUPS,
                ins=[cc_in[:]],
                outs=[cc_out[:]],
            )

            # gathered[2, 2N] -> SBUF partition-broadcast [128, 2, 2N]
            g = stats_pool.tile([128, 2, 2 * N], F32)
            cc_b = bass.AP(
                tensor=cc_out.tensor,
                offset=cc_out.offset,
                ap=[[0, 128]] + list(cc_out.ap),
            )
            nc.sync.dma_start(out=g[:], in_=cc_b)

        # ---- Phase B: issue all lfi loads up front (after fm loads in DMA
        # order). V-mean reduces run on the DVE, interleaved with the output
        # multiplies below.
        lts = []
        for u in range(U):
            lt = lfi_pool.tile([128, W, V], BF16, name=f"lt{u}", tag=f"lt{u}")
            nc.sync.dma_start(out=lt[:], in_=lfi_s[u])
            lts.append(lt)

        mlf = [
            mlf_pool.tile([128, W], BF16, name=f"mlf{u}", tag=f"mlf{u}")
            for u in range(U)
        ]

        def reduce_u(u):
            with nc.allow_low_precision(reason="bf16 V-sum of 9 values"):
                nc.vector.reduce_sum(
                    out=mlf[u][:], in_=lts[u][:], axis=mybir.AxisListType.X
                )

        reduce_u(0)

        # ---- stats finalize (waits on the collective result)
        with hp():
            s_all = stats_pool.tile([128, N], F32)
            nc.vector.tensor_add(out=s_all[:], in0=g[:, 0, 0:N], in1=g[:, 1, 0:N])
            m_all = stats_pool.tile([128, N], F32)
            nc.vector.tensor_max(
                out=m_all[:], in0=g[:, 0, N : 2 * N], in1=g[:, 1, N : 2 * N]
            )

            m9 = stats_pool.tile([128, N], F32)
            nc.vector.tensor_scalar_mul(m9[:], m_all[:], float(V))
            rec = stats_pool.tile([128, N], F32)
            nc.vector.reciprocal(out=rec[:], in_=m9[:])
            sn = stats_pool.tile([128, N], F32)
            nc.vector.tensor_mul(out=sn[:], in0=s_all[:], in1=rec[:])
            wf = stats_pool.tile([128, N], F32)
            nc.vector.tensor_mul(out=wf[:], in0=cs_sb[:], in1=sn[:])
            wf_bf = stats_pool.tile([128, N], BF16)
            nc.vector.tensor_copy(out=wf_bf[:], in_=wf[:])

        # ---- Phase B2: expand wf to wfx[y, n, x] = wf[y, n] (n-major, x
        # contiguous) by log-doubling copies (4x DVE copy mode for w >= 2).
        wfx = wfx_pool.tile([128, N, W], BF16)
        seed_dst = bass.AP(
            tensor=wfx.tensor, offset=wfx.offset, ap=[wfx.ap[0], [W, N]]
        )
        nc.vector.tensor_copy(out=seed_dst, in_=wf_bf[:])
        w = 1
        while w < W:
            nc.vector.tensor_copy(
                out=wfx[:, :, w : 2 * w], in_=wfx[:, :, 0:w]
            )
            w *= 2

        # ---- Phase C: out_s[u, y, n, x] = mlf[u][y, x] * wfx[y, n, x], with
        # the remaining V-mean reduces interleaved between output multiplies.
        # The HBM output is n-major ([U, HY, N, W]); the host transposes back
        # during unshard. n-quarter tiles keep every store's innermost HBM
        # run at 512B (full DMA rate) and pipeline at ~1MB granularity.
        NQ = 16
        def emit_tile(u, n0):
            ot = out_pool.tile([128, NQ, W], BF16, name=f"ot{u}_{n0}", tag="ot", bufs=3)
            msl = mlf[u][:]
            m_b = bass.AP(
                tensor=msl.tensor, offset=msl.offset,
                ap=[msl.ap[0], [0, NQ], msl.ap[1]],
            )
            nc.vector.tensor_mul(out=ot[:], in0=m_b, in1=wfx[:, n0 : n0 + NQ, :])
            nc.sync.dma_start(out=out_s[u, :, n0 : n0 + NQ, :], in_=ot[:])

        for u in range(U):
            for nh in range(N // NQ):
                emit_tile(u, nh * NQ)
            if u + 1 < U:
                reduce_u(u + 1)


def build_nc():
    nc = bacc.Bacc("TRN2", target_bir_lowering=False, debug=True)
    lfi_s = nc.dram_tensor("lfi_s", [U, HY, W, V], BF16, kind="ExternalInput")
    fm = nc.dram_tensor("fm", [H, HY, N], BF16, kind="ExternalInput")
    out_s = nc.dram_tensor("out_s", [U, HY, N, W], BF16, kind="ExternalOutput")
    cc_in = nc.dram_tensor("cc_in", [1, 2 * N], F32)
    cc_out = nc.dram_tensor("cc_out", [2, 2 * N], F32)
    with tile.TileContext(nc) as tc:
        build_kernel_body(nc, tc, lfi_s, fm, out_s, cc_in[:], cc_out[:])
    nc.compile()
    return nc


_CACHE = {}


def make_in_maps(lfi, f_maps):
    lfi16 = lfi.astype(bfloat16)
    fm16 = f_maps.astype(bfloat16)
    in_maps = []
    for c in range(8):
        b, half = divmod(c, 2)
        lf = np.ascontiguousarray(lfi16[b, :, half * HY : (half + 1) * HY])
        fmc = np.ascontiguousarray(fm16[b][:, half * HY : (half + 1) * HY, :])
        in_maps.append({"lfi_s": lf, "fm": fmc})
    return in_maps


def kernel(lfi, f_maps):
    lfi = np.asarray(lfi, dtype=np.float32)
    f_maps = np.asarray(f_maps, dtype=np.float32)
    if "nc" not in _CACHE:
        _CACHE["nc"] = build_nc()
    nc = _CACHE["nc"]
    res = run_bass_kernel_spmd(nc, make_in_maps(lfi, f_maps), list(range(8)))
    out = np.empty((B, U, H, W, N), np.float32)
    for c in range(8):
        b, half = divmod(c, 2)
        # device output is [U, HY, N, W]; unshard transposes back to
        # [U, HY, W, N]
        out[b, :, half * HY : (half + 1) * HY] = (
            res.results[c]["out_s"].astype(np.float32).transpose(0, 1, 3, 2)
        )
    return out
